# revision 29
# baseline (speedup 1.0000x reference)
"""Trainium2 Bass kernel for nn_Net_16174846837292 (NNConv GNN message passing).

Strategy (graph-sharded, aggregation-folded):
  pooled[g,o] = sum_{e: batch[dst[e]]=g} w_e * msg[e,o],  w_e = 1/max(cnt[dst_e],1)
  msg[e,o]    = sum_{k,i} e3[e,k]*h[src_e,i]*e4w[k,i*128+o] + sum_i h[src_e,i]*e4b[i*128+o]
  => pooled^T[o,g] = sum_k A2[k*128+i, o]^T ZG_g[i,k] + Br^T HW_g        (tiny matmuls)
     ZG_g[i,k] = sum_{e in g} (w_e h[src_e,i]) e3[e,k],  HW_g[i] = sum_{e in g} w_e h[src_e,i]
  Never materializes the per-edge [128,128] weight matrices (512 MB in the
  reference) nor any per-node [16384] intermediates.

Sharding: edges grouped by the graph of their destination node; 8 graphs per
core; in-degree weights and the x[src] gather are host-side index prep.

Performance structure (see git history for the evolution):
  - All five MLP matmul layers run as fp16 hi/lo 3-term splits
    (wh@xh + wh@xl + wl@xh, fp32 PSUM): 3 cycles/row instead of fp32's 4,
    with ~fp32 accuracy (residual term ~(2^-11)^2). Weights and the two
    input tensors are pre-split on the host; intermediate activations are
    split on gpsimd (hi cast) + vector (mixed-dtype subtract).
  - Small const blobs head both HWDGE rings, then a2h/a2l (8 MB) stream
    behind them, one half per ring (packet round-robin starves small
    transfers behind a multi-MB stream otherwise).
  - Dummy bf16 matmuls run while DMAs land: the HAM clock gate holds the
    PE at 1.2 GHz until ~17 us of sustained activity, so activity starts
    as early as possible.
  - w_e folds into the hsrc PSUM->SBUF copy (activation Copy with scale),
    ZG's moving operand carries a constant ones column for HW_g.
  - Final e4 contraction: a2 bf16 hi/lo stationary (fast weight load),
    zg hi/lo packed as one 16-column moving operand: 257 weight loads.
"""

import numpy as np
from contextlib import ExitStack

import ml_dtypes
import concourse.bass as bass
import concourse.tile as tile
from concourse import bacc, mybir
from concourse.bass_utils import run_bass_kernel_spmd

N_CORES = 8
N, E, G, H = 4096, 8192, 64, 128
NODE_DIM, EDGE_DIM = 11, 5
G_PER_CORE = G // N_CORES          # 8 graph slots per core
CAP = 192                          # edge slots per graph (64-aligned segments)
EP = G_PER_CORE * CAP              # 1536 edge slots per core
NT = EP // 128                     # 12 edge tiles per core
NCH = EP // 512                    # 3 512-wide chunks for the MLPs

# fp16 weight blob column layout (hi/lo pairs; e1 + p1 only — their moving
# data ships pre-split from the host, so the 3-term trick costs nothing)
W16_E1H, W16_E1L = 0, 128
W16_P1H, W16_P1L = 256, 384
WB16 = 512
# f32 blob column layout
R_WME, R_E1B, R_P1B, R_P2B, R_E2B, R_E3B = 0, 12, 13, 14, 15, 17
R_BR, R_IDN, R_P2W, R_E2W, R_E3W = 18, 146, 274, 402, 658
WB_F = 914

f32 = mybir.dt.float32
f16 = mybir.dt.float16
bf16 = mybir.dt.bfloat16
AF = mybir.ActivationFunctionType
OP = mybir.AluOpType


def _slot_segments(s):
    """(tile, p0, p1) segments of graph slot s in the (p, t) edge grid."""
    segs, a, end = [], s * CAP, (s + 1) * CAP
    while a < end:
        t, p0 = divmod(a, 128)
        take = min(128 - p0, end - a)
        segs.append((t, p0, p0 + take))
        a += take
    return segs


def _emit(nc, tc, io):
    es = ExitStack()
    const = es.enter_context(tc.tile_pool(name="const", bufs=1))
    big = es.enter_context(tc.tile_pool(name="big", bufs=1))
    work = es.enter_context(tc.tile_pool(name="work", bufs=3))
    psA = es.enter_context(tc.tile_pool(name="psA", bufs=2, space="PSUM"))
    psB = es.enter_context(tc.tile_pool(name="psB", bufs=2, space="PSUM"))
    psZ = es.enter_context(tc.tile_pool(name="psZ", bufs=2, space="PSUM"))
    psO = es.enter_context(tc.tile_pool(name="psO", bufs=1, space="PSUM"))

    with es:
        # ---- DMA: small consts head both rings, then the a2 halves ----------
        w16 = const.tile([128, WB16], f16, tag="w16")
        nc.sync.dma_start(w16[:], io["w16"][:])
        ea16 = const.tile([EDGE_DIM, 2 * EP], f16, tag="ea16")
        nc.scalar.dma_start(ea16[:], io["ea16"][:])
        wf = const.tile([128, WB_F], f32, tag="wf")
        nc.sync.dma_start(wf[:], io["wb_f32"][:])
        xg16 = const.tile([NODE_DIM, 2 * EP], f16, tag="xg16")
        nc.scalar.dma_start(xg16[:], io["xg16"][:])
        a2h_sb = big.tile([128, 128 * H], bf16, tag="a2h")
        nc.sync.dma_start(a2h_sb[:], io["a2h"][:])
        a2l_sb = big.tile([128, 128 * H], bf16, tag="a2l")
        nc.scalar.dma_start(a2l_sb[:], io["a2l"][:])

        wme = wf[:, R_WME:R_WME + NT]
        b_e1 = wf[:, R_E1B:R_E1B + 1]
        b_p1 = wf[:, R_P1B:R_P1B + 1]
        b_p2 = wf[:, R_P2B:R_P2B + 1]
        b_e2 = wf[:, R_E2B:R_E2B + 2]
        b_e3 = wf[:, R_E3B:R_E3B + 1]
        w_br = wf[:, R_BR:R_BR + 128]
        idn = wf[:, R_IDN:R_IDN + 128]
        w_p2 = wf[:, R_P2W:R_P2W + 128]
        w_e2 = wf[:, R_E2W:R_E2W + 256]
        w_e30 = wf[:, R_E3W:R_E3W + 128]
        w_e31 = wf[:, R_E3W + 128:R_E3W + 256]

        def w16s(c0, rows=128, w=128):
            return w16[0:rows, c0:c0 + w]

        # ---- PE warm-up while DMAs land (HAM clock gate) --------------------
        # Dense 512-col bf16 matmuls flip the clock gate to 2.4 GHz within
        # ~3 us; sparse small matmuls do not count as "busy".
        wup = const.tile([128, 512], bf16, tag="wup")
        nc.vector.memset(wup[:], 0.0)
        for r in range(12):
            pw = psA.tile([128, 512], f32, tag="mlp")
            nc.tensor.matmul(pw[:], wup[:, 0:128], wup[:], start=True,
                             stop=True)

        def mm3(ps, c_h, c_l, xh, xl, rows=128, w=128):
            nc.tensor.matmul(ps, w16s(c_h, rows, w), xh, start=True,
                             stop=False)
            nc.tensor.matmul(ps, w16s(c_h, rows, w), xl, start=False,
                             stop=False)
            nc.tensor.matmul(ps, w16s(c_l, rows, w), xh, start=False,
                             stop=True)

        # ---- edge MLP layer 1 + node MLP layer 1 (feature-major) ------------
        e1o = big.tile([128, EP], f32, tag="e1o")
        for q in range(NCH):
            ps = psA.tile([128, 512], f32, tag="mlp")
            mm3(ps[:], W16_E1H, W16_E1L,
                ea16[:, q * 512:(q + 1) * 512],
                ea16[:, EP + q * 512:EP + (q + 1) * 512], rows=EDGE_DIM)
            nc.vector.tensor_scalar(e1o[:, q * 512:(q + 1) * 512], ps[:],
                                    b_e1, 0.0, op0=OP.add, op1=OP.max)

        h1 = big.tile([128, EP], f32, tag="h1")
        for q in range(NCH):
            ps = psA.tile([128, 512], f32, tag="mlp")
            mm3(ps[:], W16_P1H, W16_P1L,
                xg16[:, q * 512:(q + 1) * 512],
                xg16[:, EP + q * 512:EP + (q + 1) * 512], rows=NODE_DIM)
            nc.scalar.activation(h1[:, q * 512:(q + 1) * 512], ps[:], AF.Relu,
                                 bias=b_p1)

        # ---- node MLP layer 2 (no relu) + transpose to edge-major -----------
        h2 = big.tile([128, EP], f32, tag="h2")
        for q in range(NCH):
            ps = psA.tile([128, 512], f32, tag="mlp")
            nc.tensor.matmul(ps[:], w_p2, h1[:, q * 512:(q + 1) * 512],
                             start=True, stop=True)
            nc.vector.tensor_scalar_add(h2[:, q * 512:(q + 1) * 512], ps[:],
                                        b_p2)
        # w_e folds into the PSUM->SBUF copy: hsrc[e,:] = w_e * h[src_e,:]
        hsrc = big.tile([128, NT, H], f32, tag="hsrc")
        for t in range(NT):
            pt = psB.tile([128, 128], f32, tag="tr")
            nc.tensor.transpose(pt[:], h2[:, t * 128:(t + 1) * 128], idn)
            nc.scalar.mul(hsrc[:, t, :], pt[:], wme[:, t:t + 1])

        # ---- edge MLP layers 2-3 (feature-major, fp32) ----------------------
        e2o0 = big.tile([128, EP], f32, tag="e2o0")
        e2o1 = big.tile([128, EP], f32, tag="e2o1")
        for m, e2o in enumerate((e2o0, e2o1)):
            for q in range(NCH):
                ps = psA.tile([128, 512], f32, tag="mlp")
                nc.tensor.matmul(ps[:], w_e2[:, m * 128:(m + 1) * 128],
                                 e1o[:, q * 512:(q + 1) * 512],
                                 start=True, stop=True)
                if m == 0:
                    nc.scalar.activation(e2o[:, q * 512:(q + 1) * 512], ps[:],
                                         AF.Relu, bias=b_e2[:, 0:1])
                else:
                    nc.vector.tensor_scalar(e2o[:, q * 512:(q + 1) * 512],
                                            ps[:], b_e2[:, 1:2], 0.0,
                                            op0=OP.add, op1=OP.max)

        e3o = big.tile([128, EP], f32, tag="e3o")
        for q in range(NCH):
            c0, c1 = q * 512, (q + 1) * 512
            ps = psA.tile([128, 512], f32, tag="mlp")
            nc.tensor.matmul(ps[:], w_e30, e2o0[:, c0:c1],
                             start=True, stop=False)
            nc.tensor.matmul(ps[:], w_e31, e2o1[:, c0:c1],
                             start=False, stop=True)
            nc.scalar.activation(e3o[:, c0:c1], ps[:], AF.Relu, bias=b_e3)

        # ---- per-tile transpose to edge-major (w_e already in hsrc) ---------
        e3x = big.tile([128, NT, H + 1], f32, tag="e3x")
        for t in range(NT):
            nc.gpsimd.memset(e3x[:, t, H:H + 1], 1.0)
            pt = psB.tile([128, 128], f32, tag="tr")
            nc.tensor.transpose(pt[:], e3o[:, t * 128:(t + 1) * 128], idn)
            nc.vector.tensor_copy(e3x[:, t, 0:H], pt[:])

        # ---- per-graph ZG accumulation + bf16 hi/lo packed [zh|zl] ----------
        zg2 = big.tile([128, 2 * G_PER_CORE, H], bf16, tag="zg2")
        hw_f = big.tile([128, G_PER_CORE], f32, tag="hwf")
        for s in range(G_PER_CORE):
            segs = _slot_segments(s)
            pz = psZ.tile([128, H + 1], f32, tag="zg")
            for n, (t, p0, p1) in enumerate(segs):
                nc.tensor.matmul(pz[:], hsrc[p0:p1, t, :],
                                 e3x[p0:p1, t, :],
                                 start=(n == 0), stop=(n == len(segs) - 1))
            nc.vector.tensor_copy(zg2[:, s, :], pz[:, 0:H])
            nc.scalar.copy(hw_f[:, s:s + 1], pz[:, H:H + 1])
            nc.vector.tensor_tensor(zg2[:, G_PER_CORE + s, :], pz[:, 0:H],
                                    zg2[:, s, :], op=OP.subtract)

        # ---- final e4 contraction -------------------------------------------
        # po16 = sum_k a2h_k @ [zh|zl]_k ; po8 = sum_k a2l_k @ zh_k + Br @ HW.
        # Separate PSUM tiles so the po16 combine overlaps the al-loop.
        po16 = psO.tile([128, 2 * G_PER_CORE], f32, tag="po16")
        for k in range(H):
            nc.tensor.matmul(po16[:], a2h_sb[:, k * 128:(k + 1) * 128],
                             zg2[:, :, k], start=(k == 0), stop=(k == H - 1))
        po8 = psO.tile([128, G_PER_CORE], f32, tag="po8")
        for k in range(H):
            nc.tensor.matmul(po8[:], a2l_sb[:, k * 128:(k + 1) * 128],
                             zg2[:, 0:G_PER_CORE, k], start=(k == 0),
                             stop=False)
        nc.tensor.matmul(po8[:], w_br, hw_f[:], start=False, stop=True)
        s1 = work.tile([128, G_PER_CORE], f32, tag="s1")
        nc.scalar.copy(s1[:], po16[:, 0:G_PER_CORE])
        osum = work.tile([128, G_PER_CORE], f32, tag="osum")
        nc.vector.tensor_tensor(osum[:], po16[:, G_PER_CORE:2 * G_PER_CORE],
                                s1[:], op=OP.add)
        ot = work.tile([128, G_PER_CORE], f32, tag="ot")
        nc.vector.tensor_tensor(ot[:], po8[:], osum[:], op=OP.add)
        # transpose to [G_PER_CORE, H] so the output DMA is 8 big descriptors
        pot = psB.tile([128, 128], f32, tag="tr")
        nc.tensor.matmul(pot[0:G_PER_CORE, :], ot[:], idn, start=True,
                         stop=True)
        otT = work.tile([G_PER_CORE, H], f32, tag="otT")
        nc.scalar.copy(otT[:], pot[0:G_PER_CORE, :])
        nc.sync.dma_start(io["pooled_t"][:, :], otT[:])


_CACHE = {}


def _build():
    if "nc" in _CACHE:
        return _CACHE["nc"]
    nc = bacc.Bacc("TRN2", target_bir_lowering=False, debug=False,
                   num_devices=N_CORES)
    io = {}

    def din(name, shape, dt=f32):
        io[name] = nc.dram_tensor(name, shape, dt, kind="ExternalInput").ap()

    din("wb_f32", [128, WB_F])
    din("w16", [128, WB16], f16)
    din("ea16", [EDGE_DIM, 2 * EP], f16)
    din("xg16", [NODE_DIM, 2 * EP], f16)
    din("a2h", [128, 128 * H], bf16)
    din("a2l", [128, 128 * H], bf16)
    io["pooled_t"] = nc.dram_tensor("pooled_t", [G_PER_CORE, H], f32,
                                    kind="ExternalOutput").ap()

    with tile.TileContext(nc) as tc:
        _emit(nc, tc, io)
    nc.compile()
    _CACHE["nc"] = nc
    return nc


def _f16pair(w):
    w = np.asarray(w, np.float32)
    hi = w.astype(np.float16)
    lo = (w - hi.astype(np.float32)).astype(np.float16)
    return hi, lo


def _host_prep(inputs):
    x = np.ascontiguousarray(np.asarray(inputs["x"], dtype=np.float32))
    ea = np.asarray(inputs["edge_attr"], dtype=np.float32)
    ei = np.asarray(inputs["edge_index"]).astype(np.int64)
    batch = np.asarray(inputs["batch"]).astype(np.int64)
    src, dst = ei[0], ei[1]
    gid = batch[dst]
    indeg = np.bincount(dst, minlength=N).astype(np.float32)
    w_edge = (np.float32(1.0)
              / np.maximum(indeg, np.float32(1.0)))[dst].astype(np.float32)

    a2f = np.ascontiguousarray(
        np.asarray(inputs["e4_w"], np.float32)
        .reshape(128, 128, 128).transpose(1, 0, 2).reshape(128, 128 * H))
    a2h = a2f.astype(ml_dtypes.bfloat16)
    a2l = (a2f - a2h.astype(np.float32)).astype(ml_dtypes.bfloat16)

    w16 = np.zeros((128, WB16), np.float16)
    e1h_, e1l_ = _f16pair(inputs["e1_w"])
    w16[0:EDGE_DIM, W16_E1H:W16_E1H + 128] = e1h_
    w16[0:EDGE_DIM, W16_E1L:W16_E1L + 128] = e1l_
    p1h_, p1l_ = _f16pair(inputs["p1_w"])
    w16[0:NODE_DIM, W16_P1H:W16_P1H + 128] = p1h_
    w16[0:NODE_DIM, W16_P1L:W16_P1L + 128] = p1l_

    wf0 = np.zeros((128, WB_F), np.float32)
    wf0[:, R_E1B] = np.asarray(inputs["e1_b"], np.float32)
    wf0[:, R_P1B] = np.asarray(inputs["p1_b"], np.float32)
    wf0[:, R_P2B] = np.asarray(inputs["p2_b"], np.float32)
    wf0[:, R_E2B:R_E2B + 2] = np.asarray(inputs["e2_b"],
                                         np.float32).reshape(2, 128).T
    wf0[:, R_E3B] = np.asarray(inputs["e3_b"], np.float32)
    wf0[:, R_BR:R_BR + 128] = np.asarray(inputs["e4_b"],
                                         np.float32).reshape(128, 128)
    wf0[:, R_IDN:R_IDN + 128] = np.eye(128, dtype=np.float32)
    wf0[:, R_P2W:R_P2W + 128] = np.asarray(inputs["p2_w"], np.float32)
    wf0[:, R_E2W:R_E2W + 256] = np.asarray(inputs["e2_w"], np.float32)
    e3w = np.asarray(inputs["e3_w"], np.float32)
    wf0[:, R_E3W:R_E3W + 128] = e3w[0:128, :]
    wf0[:, R_E3W + 128:R_E3W + 256] = e3w[128:256, :]

    com = {"w16": np.ascontiguousarray(w16), "a2h": a2h, "a2l": a2l}

    in_maps = []
    for c in range(N_CORES):
        ea_s = np.zeros((EP, EDGE_DIM), np.float32)
        xg_s = np.zeros((EP, NODE_DIM), np.float32)
        w_s = np.zeros(EP, np.float32)
        for s in range(G_PER_CORE):
            es_idx = np.where(gid == c * G_PER_CORE + s)[0]
            assert len(es_idx) <= CAP, \
                f"graph {c * G_PER_CORE + s}: {len(es_idx)} edges"
            pos = s * CAP + np.arange(len(es_idx))
            ea_s[pos] = ea[es_idx]
            xg_s[pos] = x[src[es_idx]]
            w_s[pos] = w_edge[es_idx]
        wf = wf0.copy()
        wf[:, R_WME:R_WME + NT] = w_s.reshape(NT, 128).T
        eh, el = _f16pair(ea_s.T)
        xh, xl = _f16pair(xg_s.T)
        m = dict(com)
        m["wb_f32"] = np.ascontiguousarray(wf)
        m["ea16"] = np.ascontiguousarray(np.concatenate([eh, el], axis=1))
        m["xg16"] = np.ascontiguousarray(np.concatenate([xh, xl], axis=1))
        in_maps.append(m)
    return in_maps


def _run(inputs, trace=False, tmpdir=None):
    nc = _build()
    in_maps = _host_prep(inputs)
    if trace:
        # No egress in this sandbox: neutralize the artifact upload the
        # trace path performs after NTFF capture, and register the NTFF
        # hook module if the image lacks antenv.axon_hooks.
        from concourse import bass_utils as _bu
        _bu.upload_artifacts = lambda d: d
        try:
            from antenv import axon_hooks  # noqa: F401
        except ImportError:
            import sys as _sys, types as _types
            from trn_agent_boot.trn_boot import _ntff_profile_via_ctypes
            _hook = _ntff_profile_via_ctypes("/opt/axon/libaxon_pjrt.so")
            mod = _types.ModuleType("antenv.axon_hooks")
            mod.get_axon_ntff_profile_hook = lambda: _hook
            mod.set_axon_ntff_profile_hook = lambda h: None
            _sys.modules["antenv.axon_hooks"] = mod
    res = run_bass_kernel_spmd(nc, in_maps, list(range(N_CORES)),
                               trace=trace, tmpdir=tmpdir)
    out = np.empty((G, H), np.float32)
    for c in range(N_CORES):
        out[c * G_PER_CORE:(c + 1) * G_PER_CORE, :] = res.results[c]["pooled_t"]
    return out, res


def kernel(**inputs) -> np.ndarray:
    out, _ = _run(inputs)
    return out


# revision 37
# speedup vs baseline: 1.0371x; 1.0371x over previous
"""Trainium2 Bass kernel for nn_Net_16174846837292 (NNConv GNN message passing).

Strategy (graph-sharded, aggregation-folded):
  pooled[g,o] = sum_{e: batch[dst[e]]=g} w_e * msg[e,o],  w_e = 1/max(cnt[dst_e],1)
  msg[e,o]    = sum_{k,i} e3[e,k]*h[src_e,i]*e4w[k,i*128+o] + sum_i h[src_e,i]*e4b[i*128+o]
  => pooled^T[o,g] = sum_k A2[k*128+i, o]^T ZG_g[i,k] + Br^T HW_g        (tiny matmuls)
     ZG_g[i,k] = sum_{e in g} (w_e h[src_e,i]) e3[e,k],  HW_g[i] = sum_{e in g} w_e h[src_e,i]
  Never materializes the per-edge [128,128] weight matrices (512 MB in the
  reference) nor any per-node [16384] intermediates.

Sharding: edges grouped by the graph of their destination node; 8 graphs per
core; in-degree weights and the x[src] gather are host-side index prep.

Performance structure (see git history for the evolution):
  - All five MLP matmul layers run as fp16 hi/lo 3-term splits
    (wh@xh + wh@xl + wl@xh, fp32 PSUM): 3 cycles/row instead of fp32's 4,
    with ~fp32 accuracy (residual term ~(2^-11)^2). Weights and the two
    input tensors are pre-split on the host; intermediate activations are
    split on gpsimd (hi cast) + vector (mixed-dtype subtract).
  - Small const blobs head both HWDGE rings, then a2h/a2l (8 MB) stream
    behind them, one half per ring (packet round-robin starves small
    transfers behind a multi-MB stream otherwise).
  - Dummy bf16 matmuls run while DMAs land: the HAM clock gate holds the
    PE at 1.2 GHz until ~17 us of sustained activity, so activity starts
    as early as possible.
  - w_e folds into the hsrc PSUM->SBUF copy (activation Copy with scale),
    ZG's moving operand carries a constant ones column for HW_g.
  - Final e4 contraction: a2 bf16 hi/lo stationary (fast weight load),
    zg hi/lo packed as one 16-column moving operand: 257 weight loads.
"""

import numpy as np
from contextlib import ExitStack

import ml_dtypes
import concourse.bass as bass
import concourse.tile as tile
from concourse import bacc, mybir
from concourse.bass_utils import run_bass_kernel_spmd

N_CORES = 8
N, E, G, H = 4096, 8192, 64, 128
NODE_DIM, EDGE_DIM = 11, 5
G_PER_CORE = G // N_CORES          # 8 graph slots per core
CAP = 192                          # edge slots per graph (64-aligned segments)
EP = G_PER_CORE * CAP              # 1536 edge slots per core
NT = EP // 128                     # 12 edge tiles per core
NCH = EP // 512                    # 3 512-wide chunks for the MLPs

# fp16 weight blob column layout (hi/lo pairs; e1 + p1 only — their moving
# data ships pre-split from the host, so the 3-term trick costs nothing)
W16_E1H, W16_E1L = 0, 128
W16_P1H, W16_P1L = 256, 384
WB16 = 512
# f32 blob column layout
R_WME, R_E1B, R_P1B, R_P2B, R_E2B, R_E3B = 0, 12, 13, 14, 15, 17
R_BR, R_IDN, R_P2W, R_E2W, R_E3W = 18, 146, 274, 402, 658
WB_F = 914

f32 = mybir.dt.float32
f16 = mybir.dt.float16
bf16 = mybir.dt.bfloat16
AF = mybir.ActivationFunctionType
OP = mybir.AluOpType


def _slot_segments(s):
    """(tile, p0, p1) segments of graph slot s in the (p, t) edge grid."""
    segs, a, end = [], s * CAP, (s + 1) * CAP
    while a < end:
        t, p0 = divmod(a, 128)
        take = min(128 - p0, end - a)
        segs.append((t, p0, p0 + take))
        a += take
    return segs


def _emit(nc, tc, io):
    es = ExitStack()
    const = es.enter_context(tc.tile_pool(name="const", bufs=1))
    big = es.enter_context(tc.tile_pool(name="big", bufs=1))
    work = es.enter_context(tc.tile_pool(name="work", bufs=3))
    psA = es.enter_context(tc.tile_pool(name="psA", bufs=2, space="PSUM"))
    psB = es.enter_context(tc.tile_pool(name="psB", bufs=2, space="PSUM"))
    psZ = es.enter_context(tc.tile_pool(name="psZ", bufs=3, space="PSUM"))
    psO = es.enter_context(tc.tile_pool(name="psO", bufs=1, space="PSUM"))

    with es:
        # ---- DMA: small consts head both rings, then the a2 halves ----------
        # wbias first: MLP activations must never stall on their bias, or the
        # resulting PE gap re-arms the HAM throttle.
        wbias = const.tile([128, 6], f32, tag="wbias")
        nc.sync.dma_start(wbias[:], io["wbias"][:])
        w16 = const.tile([128, WB16], f16, tag="w16")
        nc.sync.dma_start(w16[:], io["w16"][:])
        ea16 = const.tile([EDGE_DIM, 2 * EP], f16, tag="ea16")
        nc.scalar.dma_start(ea16[:], io["ea16"][:])
        wf = const.tile([128, WB_F], f32, tag="wf")
        nc.sync.dma_start(wf[:], io["wb_f32"][:])
        xg16 = const.tile([NODE_DIM, 2 * EP], f16, tag="xg16")
        nc.scalar.dma_start(xg16[:], io["xg16"][:])
        a2h_sb = big.tile([128, 128 * H], bf16, tag="a2h")
        nc.sync.dma_start(a2h_sb[:], io["a2h"][:])
        a2l_sb = big.tile([128, 128 * H], bf16, tag="a2l")
        nc.scalar.dma_start(a2l_sb[:], io["a2l"][:])

        wme = wf[:, R_WME:R_WME + NT]
        b_e1 = wbias[:, 0:1]
        b_p1 = wbias[:, 1:2]
        b_p2 = wbias[:, 2:3]
        b_e2 = wbias[:, 3:5]
        b_e3 = wbias[:, 5:6]
        w_br = wf[:, R_BR:R_BR + 128]
        idn = wf[:, R_IDN:R_IDN + 128]
        w_p2 = wf[:, R_P2W:R_P2W + 128]
        w_e2 = wf[:, R_E2W:R_E2W + 256]
        w_e30 = wf[:, R_E3W:R_E3W + 128]
        w_e31 = wf[:, R_E3W + 128:R_E3W + 256]

        def w16s(c0, rows=128, w=128):
            return w16[0:rows, c0:c0 + w]

        # ---- PE warm-up while DMAs land (HAM clock gate) --------------------
        # Dense 512-col bf16 matmuls flip the clock gate to 2.4 GHz within
        # ~3 us; sparse small matmuls do not count as "busy".
        wup = const.tile([128, 512], bf16, tag="wup")
        nc.vector.memset(wup[:], 0.0)
        for r in range(10):
            pw = psA.tile([128, 512], f32, tag="mlp")
            nc.tensor.matmul(pw[:], wup[:, 0:128], wup[:], start=True,
                             stop=True)

        def mm3(ps, c_h, c_l, xh, xl, rows=128, w=128):
            nc.tensor.matmul(ps, w16s(c_h, rows, w), xh, start=True,
                             stop=False)
            nc.tensor.matmul(ps, w16s(c_h, rows, w), xl, start=False,
                             stop=False)
            nc.tensor.matmul(ps, w16s(c_l, rows, w), xh, start=False,
                             stop=True)

        # ---- edge MLP layer 1 + node MLP layer 1 (feature-major) ------------
        e1o = big.tile([128, EP], f32, tag="e1o")
        for q in range(NCH):
            ps = psA.tile([128, 512], f32, tag="mlp")
            mm3(ps[:], W16_E1H, W16_E1L,
                ea16[:, q * 512:(q + 1) * 512],
                ea16[:, EP + q * 512:EP + (q + 1) * 512], rows=EDGE_DIM)
            nc.vector.tensor_scalar(e1o[:, q * 512:(q + 1) * 512], ps[:],
                                    b_e1, 0.0, op0=OP.add, op1=OP.max)

        h1 = big.tile([128, EP], f32, tag="h1")
        for q in range(NCH):
            ps = psA.tile([128, 512], f32, tag="mlp")
            mm3(ps[:], W16_P1H, W16_P1L,
                xg16[:, q * 512:(q + 1) * 512],
                xg16[:, EP + q * 512:EP + (q + 1) * 512], rows=NODE_DIM)
            nc.scalar.activation(h1[:, q * 512:(q + 1) * 512], ps[:], AF.Relu,
                                 bias=b_p1)

        # ---- node MLP layer 2 (no relu) + transpose to edge-major -----------
        h2 = big.tile([128, EP], f32, tag="h2")
        for q in range(NCH):
            ps = psA.tile([128, 512], f32, tag="mlp")
            nc.tensor.matmul(ps[:], w_p2, h1[:, q * 512:(q + 1) * 512],
                             start=True, stop=True)
            nc.vector.tensor_scalar_add(h2[:, q * 512:(q + 1) * 512], ps[:],
                                        b_p2)
        # w_e folds into the PSUM->SBUF copy: hsrc[e,:] = w_e * h[src_e,:]
        hsrc = big.tile([128, NT, H], f32, tag="hsrc")
        for t in range(NT):
            pt = psB.tile([128, 128], f32, tag="tr")
            nc.tensor.transpose(pt[:], h2[:, t * 128:(t + 1) * 128], idn)
            nc.scalar.mul(hsrc[:, t, :], pt[:], wme[:, t:t + 1])

        # ---- edge MLP layers 2-3 (feature-major, fp32) ----------------------
        e2o0 = big.tile([128, EP], f32, tag="e2o0")
        e2o1 = big.tile([128, EP], f32, tag="e2o1")
        for m, e2o in enumerate((e2o0, e2o1)):
            for q in range(NCH):
                ps = psA.tile([128, 512], f32, tag="mlp")
                nc.tensor.matmul(ps[:], w_e2[:, m * 128:(m + 1) * 128],
                                 e1o[:, q * 512:(q + 1) * 512],
                                 start=True, stop=True)
                if m == 0:
                    nc.scalar.activation(e2o[:, q * 512:(q + 1) * 512], ps[:],
                                         AF.Relu, bias=b_e2[:, 0:1])
                else:
                    nc.vector.tensor_scalar(e2o[:, q * 512:(q + 1) * 512],
                                            ps[:], b_e2[:, 1:2], 0.0,
                                            op0=OP.add, op1=OP.max)

        e3o = big.tile([128, EP], f32, tag="e3o")
        for q in range(NCH):
            c0, c1 = q * 512, (q + 1) * 512
            ps = psA.tile([128, 512], f32, tag="mlp")
            nc.tensor.matmul(ps[:], w_e30, e2o0[:, c0:c1],
                             start=True, stop=False)
            nc.tensor.matmul(ps[:], w_e31, e2o1[:, c0:c1],
                             start=False, stop=True)
            nc.scalar.activation(e3o[:, c0:c1], ps[:], AF.Relu, bias=b_e3)

        # ---- per-tile transpose to edge-major (w_e already in hsrc) ---------
        e3x = big.tile([128, NT, H + 1], f32, tag="e3x")
        for t in range(NT):
            nc.gpsimd.memset(e3x[:, t, H:H + 1], 1.0)
            pt = psB.tile([128, 128], f32, tag="tr")
            nc.tensor.transpose(pt[:], e3o[:, t * 128:(t + 1) * 128], idn)
            nc.vector.tensor_copy(e3x[:, t, 0:H], pt[:])

        # ---- per-graph ZG accumulation + bf16 hi/lo packed [zh|zl] ----------
        zg2 = big.tile([128, 2 * G_PER_CORE, H], bf16, tag="zg2")
        hw_f = big.tile([128, G_PER_CORE], f32, tag="hwf")
        for s in range(G_PER_CORE):
            segs = _slot_segments(s)
            pz = psZ.tile([128, H + 1], f32, tag="zg")
            for n, (t, p0, p1) in enumerate(segs):
                nc.tensor.matmul(pz[:], hsrc[p0:p1, t, :],
                                 e3x[p0:p1, t, :],
                                 start=(n == 0), stop=(n == len(segs) - 1))
            nc.vector.tensor_copy(zg2[:, s, :], pz[:, 0:H])
            nc.scalar.copy(hw_f[:, s:s + 1], pz[:, H:H + 1])
            nc.vector.tensor_tensor(zg2[:, G_PER_CORE + s, :], pz[:, 0:H],
                                    zg2[:, s, :], op=OP.subtract)

        # ---- final e4 contraction -------------------------------------------
        # po[:,0:16] = sum_k a2h_k @ [zh|zl]_k
        # po[:,16:24] = sum_k a2l_k @ zh_k + Br @ HW   (same PSUM bank)
        po = psO.tile([128, 3 * G_PER_CORE], f32, tag="po")
        for k in range(H):
            nc.tensor.matmul(po[:, 0:2 * G_PER_CORE],
                             a2h_sb[:, k * 128:(k + 1) * 128],
                             zg2[:, :, k], start=(k == 0), stop=(k == H - 1))
        for k in range(H):
            nc.tensor.matmul(po[:, 2 * G_PER_CORE:3 * G_PER_CORE],
                             a2l_sb[:, k * 128:(k + 1) * 128],
                             zg2[:, 0:G_PER_CORE, k], start=(k == 0),
                             stop=False)
        nc.tensor.matmul(po[:, 2 * G_PER_CORE:3 * G_PER_CORE], w_br, hw_f[:],
                         start=False, stop=True)
        s1 = work.tile([128, G_PER_CORE], f32, tag="s1")
        nc.scalar.copy(s1[:], po[:, 0:G_PER_CORE])
        osum = work.tile([128, G_PER_CORE], f32, tag="osum")
        nc.vector.tensor_tensor(osum[:], po[:, G_PER_CORE:2 * G_PER_CORE],
                                s1[:], op=OP.add)
        ot = work.tile([128, G_PER_CORE], f32, tag="ot")
        nc.vector.tensor_tensor(ot[:], po[:, 2 * G_PER_CORE:3 * G_PER_CORE],
                                osum[:], op=OP.add)
        # transpose to [G_PER_CORE, H] so the output DMA is 8 big descriptors
        pot = psB.tile([128, 128], f32, tag="tr")
        nc.tensor.matmul(pot[0:G_PER_CORE, :], ot[:], idn, start=True,
                         stop=True)
        otT = work.tile([G_PER_CORE, H], f32, tag="otT")
        nc.scalar.copy(otT[:], pot[0:G_PER_CORE, :])
        nc.sync.dma_start(io["pooled_t"][:, :], otT[:])


_CACHE = {}


def _build():
    if "nc" in _CACHE:
        return _CACHE["nc"]
    nc = bacc.Bacc("TRN2", target_bir_lowering=False, debug=False,
                   num_devices=N_CORES)
    io = {}

    def din(name, shape, dt=f32):
        io[name] = nc.dram_tensor(name, shape, dt, kind="ExternalInput").ap()

    din("wbias", [128, 6])
    din("wb_f32", [128, WB_F])
    din("w16", [128, WB16], f16)
    din("ea16", [EDGE_DIM, 2 * EP], f16)
    din("xg16", [NODE_DIM, 2 * EP], f16)
    din("a2h", [128, 128 * H], bf16)
    din("a2l", [128, 128 * H], bf16)
    io["pooled_t"] = nc.dram_tensor("pooled_t", [G_PER_CORE, H], f32,
                                    kind="ExternalOutput").ap()

    with tile.TileContext(nc) as tc:
        _emit(nc, tc, io)
    nc.compile()
    _CACHE["nc"] = nc
    return nc


def _f16pair(w):
    w = np.asarray(w, np.float32)
    hi = w.astype(np.float16)
    lo = (w - hi.astype(np.float32)).astype(np.float16)
    return hi, lo


def _host_prep(inputs):
    x = np.ascontiguousarray(np.asarray(inputs["x"], dtype=np.float32))
    ea = np.asarray(inputs["edge_attr"], dtype=np.float32)
    ei = np.asarray(inputs["edge_index"]).astype(np.int64)
    batch = np.asarray(inputs["batch"]).astype(np.int64)
    src, dst = ei[0], ei[1]
    gid = batch[dst]
    indeg = np.bincount(dst, minlength=N).astype(np.float32)
    w_edge = (np.float32(1.0)
              / np.maximum(indeg, np.float32(1.0)))[dst].astype(np.float32)

    a2f = np.ascontiguousarray(
        np.asarray(inputs["e4_w"], np.float32)
        .reshape(128, 128, 128).transpose(1, 0, 2).reshape(128, 128 * H))
    a2h = a2f.astype(ml_dtypes.bfloat16)
    a2l = (a2f - a2h.astype(np.float32)).astype(ml_dtypes.bfloat16)

    w16 = np.zeros((128, WB16), np.float16)
    e1h_, e1l_ = _f16pair(inputs["e1_w"])
    w16[0:EDGE_DIM, W16_E1H:W16_E1H + 128] = e1h_
    w16[0:EDGE_DIM, W16_E1L:W16_E1L + 128] = e1l_
    p1h_, p1l_ = _f16pair(inputs["p1_w"])
    w16[0:NODE_DIM, W16_P1H:W16_P1H + 128] = p1h_
    w16[0:NODE_DIM, W16_P1L:W16_P1L + 128] = p1l_

    wbias = np.zeros((128, 6), np.float32)
    wbias[:, 0] = np.asarray(inputs["e1_b"], np.float32)
    wbias[:, 1] = np.asarray(inputs["p1_b"], np.float32)
    wbias[:, 2] = np.asarray(inputs["p2_b"], np.float32)
    wbias[:, 3:5] = np.asarray(inputs["e2_b"], np.float32).reshape(2, 128).T
    wbias[:, 5] = np.asarray(inputs["e3_b"], np.float32)

    wf0 = np.zeros((128, WB_F), np.float32)
    wf0[:, R_BR:R_BR + 128] = np.asarray(inputs["e4_b"],
                                         np.float32).reshape(128, 128)
    wf0[:, R_IDN:R_IDN + 128] = np.eye(128, dtype=np.float32)
    wf0[:, R_P2W:R_P2W + 128] = np.asarray(inputs["p2_w"], np.float32)
    wf0[:, R_E2W:R_E2W + 256] = np.asarray(inputs["e2_w"], np.float32)
    e3w = np.asarray(inputs["e3_w"], np.float32)
    wf0[:, R_E3W:R_E3W + 128] = e3w[0:128, :]
    wf0[:, R_E3W + 128:R_E3W + 256] = e3w[128:256, :]

    com = {"w16": np.ascontiguousarray(w16), "a2h": a2h, "a2l": a2l,
           "wbias": np.ascontiguousarray(wbias)}

    in_maps = []
    for c in range(N_CORES):
        ea_s = np.zeros((EP, EDGE_DIM), np.float32)
        xg_s = np.zeros((EP, NODE_DIM), np.float32)
        w_s = np.zeros(EP, np.float32)
        for s in range(G_PER_CORE):
            es_idx = np.where(gid == c * G_PER_CORE + s)[0]
            assert len(es_idx) <= CAP, \
                f"graph {c * G_PER_CORE + s}: {len(es_idx)} edges"
            pos = s * CAP + np.arange(len(es_idx))
            ea_s[pos] = ea[es_idx]
            xg_s[pos] = x[src[es_idx]]
            w_s[pos] = w_edge[es_idx]
        wf = wf0.copy()
        wf[:, R_WME:R_WME + NT] = w_s.reshape(NT, 128).T
        eh, el = _f16pair(ea_s.T)
        xh, xl = _f16pair(xg_s.T)
        m = dict(com)
        m["wb_f32"] = np.ascontiguousarray(wf)
        m["ea16"] = np.ascontiguousarray(np.concatenate([eh, el], axis=1))
        m["xg16"] = np.ascontiguousarray(np.concatenate([xh, xl], axis=1))
        in_maps.append(m)
    return in_maps


def _run(inputs, trace=False, tmpdir=None):
    nc = _build()
    in_maps = _host_prep(inputs)
    if trace:
        # No egress in this sandbox: neutralize the artifact upload the
        # trace path performs after NTFF capture, and register the NTFF
        # hook module if the image lacks antenv.axon_hooks.
        from concourse import bass_utils as _bu
        _bu.upload_artifacts = lambda d: d
        try:
            from antenv import axon_hooks  # noqa: F401
        except ImportError:
            import sys as _sys, types as _types
            from trn_agent_boot.trn_boot import _ntff_profile_via_ctypes
            _hook = _ntff_profile_via_ctypes("/opt/axon/libaxon_pjrt.so")
            mod = _types.ModuleType("antenv.axon_hooks")
            mod.get_axon_ntff_profile_hook = lambda: _hook
            mod.set_axon_ntff_profile_hook = lambda h: None
            _sys.modules["antenv.axon_hooks"] = mod
    res = run_bass_kernel_spmd(nc, in_maps, list(range(N_CORES)),
                               trace=trace, tmpdir=tmpdir)
    out = np.empty((G, H), np.float32)
    for c in range(N_CORES):
        out[c * G_PER_CORE:(c + 1) * G_PER_CORE, :] = res.results[c]["pooled_t"]
    return out, res


def kernel(**inputs) -> np.ndarray:
    out, _ = _run(inputs)
    return out
